# revision 2
# baseline (speedup 1.0000x reference)
"""GCNConv Trainium2 kernel: out = A_norm @ (X @ W) == (A_norm @ X) @ W.

Self-contained: shards the graph across 8 NeuronCores (1D row partition of
destination rows), runs a Bass/Tile kernel per core via
bass_utils.run_bass_kernel_spmd, and reassembles the full output.

Per-core device kernel:
  - X (full [n_nodes, 128] fp16) stays in DRAM; source rows are gathered with
    the dma_gather custom DMA (int16 indices) from nsub subtable slices of
    <32768 rows each.
  - Destination tiles = 128 rows. Groups of `gt` tiles share one gather call
    per subtable pass; each pass gets its own SBUF slab so the passes'
    transfers overlap.
  - Segment-sum on the PE: psumT[:, r0:r0+w] += G_chunk.T @ rhs_seg, where
    rhs_seg [128, w] holds degrees[e] at (slot, local_row[e]); rhs is built
    on-chip as (rowloc == iota) * deg from two small per-edge tensors.
  - psumT is Y^T [feat, row]; cast to fp16 it becomes the stationary operand
    of the final weights matmul: out_tile = (Y^T).T @ W.

The program structure is rebuilt per input (data-dependent segment windows)
but is identical across the 8 cores: per-(group,pass) stream lengths are
padded to the max over cores and segment windows are the union over cores.
"""

import numpy as np
from dataclasses import dataclass, field

import concourse.bass as bass
import concourse.bacc as bacc
import concourse.tile as tile
from concourse import mybir
from concourse.bass_utils import run_bass_kernel_spmd
from concourse.vector_clock import ScopedClock

F16 = mybir.dt.float16
F32 = mybir.dt.float32
I16 = mybir.dt.int16

WIN = 32          # psum column window per segment
SENT = 1024.0     # rowloc sentinel (never equals iota 0..WIN-1)
NCORES = 8
GT = 4            # tiles per gather group
NSUB = 4          # subtable passes (int16 index range)

# ---------------------------------------------------------------------------
# Patch TileContext for walrus builds that reject >1 sync-wait/instruction.
# ---------------------------------------------------------------------------

_orig_commit = tile.TileContext._commit_instruction


def _commit_patched(self, inst, lazy_reg_writes: bool = True):
    si = getattr(inst, "sync_info", None)
    if (si is not None and si.on_wait and len(si.on_wait) > 1
            and inst.engine != mybir.EngineType.Unassigned):
        waits = list(si.on_wait)
        imm = [w for w in waits if w.wait_mode == "sem-ge-imm"]
        other = [w for w in waits if w.wait_mode != "sem-ge-imm"]
        assert len(other) <= 1, f"cannot split reg-waits: {waits}"
        keep = other if other else imm[:1]
        hoist = imm if other else imm[1:]
        inst.sync_info = mybir.SyncInfo(on_wait=list(keep),
                                        on_update=list(si.on_update or []))
        for w in hoist:
            nop = mybir.InstNoOp(name=self.nc.get_next_instruction_name(),
                                 ins=[], outs=[])
            nop.engine = inst.engine
            nop.bass_nofuse = True
            nop.sync_info = mybir.SyncInfo(on_wait=[w], on_update=[])
            _orig_commit(self, nop, lazy_reg_writes=False)
    return _orig_commit(self, inst, lazy_reg_writes)


def _drain_and_barrier_patched(self, tick_clock, wait_clock):
    nc = self.nc
    probe = nc.sync.nop(nofuse=True)
    wait_clock.add_sem_waits(probe.ins, ScopedClock({None: tick_clock.global_clock}))
    si = probe.ins.sync_info
    waits = list(si.on_wait) if si is not None and si.on_wait else []
    if waits:
        probe.ins.sync_info = mybir.SyncInfo(on_wait=waits[:1], on_update=[])
        for w in waits[1:]:
            n = nc.sync.nop(nofuse=True)
            n.ins.sync_info = mybir.SyncInfo(on_wait=[w], on_update=[])
    nc.sync.drain()
    nc.all_engine_barrier()
    assert self.sems is not None
    popped = nc._tile_sem_poison_stack.pop()
    assert popped is self._sem_poison
    nc.clear_and_free_semaphores(list(self.sems.allocated().values()))
    nc.all_engine_barrier()


tile.TileContext._commit_instruction = _commit_patched
tile.TileContext._drain_and_barrier = _drain_and_barrier_patched

# ---------------------------------------------------------------------------
# Host-side prep
# ---------------------------------------------------------------------------


@dataclass
class Seg:
    pass_id: int
    chunk: int
    r0: int
    w: int
    scol: int = -1


@dataclass
class Call:
    pass_id: int
    idx_ofs: int
    n_idx: int


@dataclass
class Group:
    calls: list = field(default_factory=list)
    tiles: list = field(default_factory=list)
    segs: dict = field(default_factory=dict)
    idx_ofs: int = 0
    idx_w: int = 0


def shard_edges(row_pointers, column_index, degrees, ncores=NCORES):
    rp = row_pointers.astype(np.int64)
    n_total_rows = rp.shape[0] - 1
    rows_per_core = (n_total_rows + ncores - 1) // ncores
    n_edges = column_index.shape[0]
    # reference semantics: rows = clip(searchsorted(rp, e, 'right') - 1,
    # 0, n_rows - 1); equivalently below (incl. clip of the tails).
    edge_row = np.minimum(
        np.searchsorted(rp[1:], np.arange(n_edges), side="right"),
        n_total_rows - 1)
    cores = []
    for r in range(ncores):
        r_lo = min(r * rows_per_core, n_total_rows)
        r_hi = min(r_lo + rows_per_core, n_total_rows)
        e_lo, e_hi = np.searchsorted(edge_row, [r_lo, r_hi])
        cores.append(((edge_row[e_lo:e_hi] - r_lo).astype(np.int64),
                      column_index[e_lo:e_hi].astype(np.int64),
                      degrees[e_lo:e_hi].astype(np.float32)))
    return cores, rows_per_core, n_total_rows


def prep_all(cores_edges, n_rows_core, n_nodes, gt=GT, nsub=NSUB):
    ncores = len(cores_edges)
    sub = (n_nodes + nsub - 1) // nsub
    assert sub <= 32767, "subtable must fit int16 indexing"
    n_tiles = (n_rows_core + 127) // 128

    tile_edges = []
    for er, ec, ed in cores_edges:
        e_ofs = np.searchsorted(er, np.arange(0, n_tiles * 128 + 1, 128))
        te = []
        for t in range(n_tiles):
            lo, hi = e_ofs[t], e_ofs[t + 1]
            te.append((er[lo:hi] - t * 128, ec[lo:hi], ed[lo:hi]))
        tile_edges.append(te)

    groups = []
    idx_streams = [[] for _ in range(ncores)]
    row_cols = [[] for _ in range(ncores)]
    deg_cols = [[] for _ in range(ncores)]
    idx_cursor = 0
    scol = 0
    max_pass_chunks = [0] * nsub

    for g0 in range(0, n_tiles, gt):
        g = Group()
        g.tiles = list(range(g0, min(g0 + gt, n_tiles)))
        g.idx_ofs = idx_cursor
        tile_seglists = {t: [] for t in g.tiles}
        for c in range(nsub):
            core_idx, core_row, core_deg, core_til = [], [], [], []
            for k in range(ncores):
                si, sr, sd, st = [], [], [], []
                for t in g.tiles:
                    rl, cl, dl = tile_edges[k][t]
                    m = (cl // sub) == c
                    si.append((cl[m] - c * sub).astype(np.int16))
                    sr.append(rl[m].astype(np.float32))
                    sd.append(dl[m].astype(np.float32))
                    st.append(np.full(int(m.sum()), t, np.int32))
                core_idx.append(np.concatenate(si))
                core_row.append(np.concatenate(sr))
                core_deg.append(np.concatenate(sd))
                core_til.append(np.concatenate(st))
            P = max(ci.size for ci in core_idx)
            P = ((P + 127) // 128) * 128
            if P == 0:
                continue
            nch = P // 128
            max_pass_chunks[c] = max(max_pass_chunks[c], nch)
            for k in range(ncores):
                pad = P - core_idx[k].size
                core_idx[k] = np.concatenate([core_idx[k], np.zeros(pad, np.int16)])
                core_row[k] = np.concatenate([core_row[k], np.full(pad, SENT, np.float32)])
                core_deg[k] = np.concatenate([core_deg[k], np.zeros(pad, np.float32)])
                core_til[k] = np.concatenate([core_til[k], np.full(pad, -1, np.int32)])
                idx_streams[k].append(np.ascontiguousarray(core_idx[k].reshape(-1, 16).T))
            g.calls.append(Call(pass_id=c, idx_ofs=idx_cursor, n_idx=P))
            idx_cursor += P // 16
            for j in range(nch):
                sl = slice(j * 128, (j + 1) * 128)
                tiles_here = set()
                for k in range(ncores):
                    th = core_til[k][sl]
                    tiles_here.update(np.unique(th[th >= 0]).tolist())
                for t in sorted(tiles_here):
                    rmin, rmax = 128, -1
                    for k in range(ncores):
                        mk = core_til[k][sl] == t
                        if mk.any():
                            rk = core_row[k][sl][mk]
                            rmin = min(rmin, int(rk.min()))
                            rmax = max(rmax, int(rk.max()))
                    r0 = rmin
                    while r0 <= rmax:
                        w = min(WIN, 128 - r0)
                        cols = []
                        any_core = False
                        for k in range(ncores):
                            rj = core_row[k][sl]
                            mw = ((core_til[k][sl] == t) & (rj >= r0)
                                  & (rj < r0 + w))
                            rc = np.full(128, SENT, np.float32)
                            dc = np.zeros(128, np.float32)
                            if mw.any():
                                any_core = True
                                rc[mw] = rj[mw] - r0
                                dc[mw] = core_deg[k][sl][mw]
                            cols.append((rc, dc))
                        if any_core:
                            tile_seglists[t].append(
                                (Seg(pass_id=c, chunk=j, r0=r0, w=w), cols))
                        r0 += w
        g.idx_w = idx_cursor - g.idx_ofs
        for t in g.tiles:
            segl = []
            for seg, cols in tile_seglists[t]:
                seg.scol = scol
                scol += 1
                for k in range(ncores):
                    row_cols[k].append(cols[k][0])
                    deg_cols[k].append(cols[k][1])
                segl.append(seg)
            g.segs[t] = segl
        groups.append(g)

    arrays = []
    for k in range(ncores):
        idx_dram = (np.concatenate(idx_streams[k], axis=1)
                    if idx_streams[k] else np.zeros((16, 1), np.int16))
        idx_dram = np.ascontiguousarray(np.tile(idx_dram, (8, 1)))
        rowloc = (np.stack(row_cols[k], axis=1).astype(np.float16)
                  if row_cols[k] else np.full((128, 1), SENT, np.float16))
        degseg = (np.stack(deg_cols[k], axis=1).astype(np.float16)
                  if deg_cols[k] else np.zeros((128, 1), np.float16))
        arrays.append(dict(idx=idx_dram, rowloc=rowloc, degseg=degseg))

    meta = dict(groups=groups, n_tiles=n_tiles, nsub=nsub, sub=sub,
                idx_w=max(idx_cursor, 1), n_segs=max(scol, 1),
                pass_chunks=max_pass_chunks,
                max_iw=max((g.idx_w for g in groups), default=1),
                max_tile_segs=max((len(s) for g in groups
                                   for s in g.segs.values()), default=1))
    return meta, arrays


# ---------------------------------------------------------------------------
# Device program
# ---------------------------------------------------------------------------


def build_gcn(meta, n_nodes, d=128, g_bufs=2, num_devices=NCORES, repeats=1):
    groups = meta["groups"]
    sub = meta["sub"]
    SMAXT = max(meta["max_tile_segs"], 1)

    nc = bacc.Bacc("TRN2", target_bir_lowering=False, debug=False,
                   num_devices=num_devices)

    x = nc.dram_tensor("x", [n_nodes, d], F16, kind="ExternalInput")
    w = nc.dram_tensor("w", [d, d], F16, kind="ExternalInput")
    idxd = nc.dram_tensor("idx", [128, meta["idx_w"]], I16, kind="ExternalInput")
    rowlocd = nc.dram_tensor("rowloc", [128, meta["n_segs"]], F16,
                             kind="ExternalInput")
    degsegd = nc.dram_tensor("degseg", [128, meta["n_segs"]], F16,
                             kind="ExternalInput")
    out = nc.dram_tensor("out", [meta["n_tiles"] * 128, d], F16,
                         kind="ExternalOutput")

    with tile.TileContext(nc) as tc:
        with (
            tc.tile_pool(name="static", bufs=1) as spool,
            tc.tile_pool(name="g", bufs=g_bufs) as gpool,
            tc.tile_pool(name="idxp", bufs=g_bufs) as ipool,
            tc.tile_pool(name="rhs", bufs=3) as rpool,
            tc.tile_pool(name="psum", bufs=2, space="PSUM") as ppool,
            tc.tile_pool(name="small", bufs=3) as smpool,
        ):
            rowloc_sb = spool.tile([128, meta["n_segs"]], F16)
            nc.sync.dma_start(rowloc_sb[:, :], rowlocd[:, :])
            degseg_sb = spool.tile([128, meta["n_segs"]], F16)
            nc.sync.dma_start(degseg_sb[:, :], degsegd[:, :])
            w_sb = spool.tile([128, d], F16)
            nc.sync.dma_start(w_sb[:, :], w[:, :])
            zero_sb = spool.tile([128, d], F16)
            nc.vector.memset(zero_sb[:, :], 0.0)
            iota_sb = spool.tile([128, SMAXT * WIN], F16)
            nc.gpsimd.iota(iota_sb[:, :], pattern=[[0, SMAXT], [1, WIN]],
                           base=0, channel_multiplier=0,
                           allow_small_or_imprecise_dtypes=True)

            for _rep in range(repeats):
                for g in groups:
                    slabs = {}
                    for c in g.calls:
                        slabs[c.pass_id] = gpool.tile(
                            [128, meta["pass_chunks"][c.pass_id] * 128], F16,
                            name=f"gslab{c.pass_id}", tag=f"g{c.pass_id}")
                    islab = ipool.tile([128, meta["max_iw"]], I16, tag="i")
                    if g.idx_w:
                        nc.sync.dma_start(islab[:, :g.idx_w],
                                          idxd[:, g.idx_ofs:g.idx_ofs + g.idx_w])
                    for c in g.calls:
                        nch = c.n_idx // 128
                        out_ap = slabs[c.pass_id][:, :nch * 128]
                        lofs = c.idx_ofs - g.idx_ofs
                        nc.gpsimd.dma_gather(
                            out_ap=out_ap.rearrange("p (c d) -> p c d", d=d),
                            in_ap=x[c.pass_id * sub:
                                    min((c.pass_id + 1) * sub, n_nodes), :],
                            idxs_ap=islab[:, lofs:lofs + c.n_idx // 16],
                            num_idxs=c.n_idx,
                            num_idxs_reg=c.n_idx,
                            elem_size=d,
                            single_packet=False,
                        )
                    for t in g.tiles:
                        segs = g.segs.get(t, [])
                        ns = len(segs)
                        psum_t = ppool.tile([128, 128], F32, space="PSUM",
                                            tag="pT")
                        nc.tensor.matmul(out=psum_t[:, :], lhsT=zero_sb[:, :],
                                         rhs=zero_sb[:, :], start=True,
                                         stop=(ns == 0), skip_group_check=True)
                        if ns:
                            s0 = segs[0].scol
                            rhs = rpool.tile([128, ns * WIN], F16, tag="rhs")
                            nc.vector.tensor_tensor(
                                out=rhs[:, :],
                                in0=rowloc_sb[:, s0:s0 + ns].to_broadcast(
                                    [128, ns, WIN]),
                                in1=iota_sb[:, :ns * WIN],
                                op=mybir.AluOpType.is_equal,
                            )
                            rhs2 = rpool.tile([128, ns * WIN], F16, tag="rhs2")
                            nc.vector.tensor_tensor(
                                out=rhs2[:, :],
                                in0=rhs[:, :],
                                in1=degseg_sb[:, s0:s0 + ns].to_broadcast(
                                    [128, ns, WIN]),
                                op=mybir.AluOpType.mult,
                            )
                            for i, s in enumerate(segs):
                                nc.tensor.matmul(
                                    out=psum_t[:, s.r0:s.r0 + s.w],
                                    lhsT=slabs[s.pass_id][:, s.chunk * 128:
                                                          (s.chunk + 1) * 128],
                                    rhs=rhs2[:, i * WIN:i * WIN + s.w],
                                    start=False, stop=(i == ns - 1),
                                    skip_group_check=True,
                                )
                        yt = smpool.tile([128, 128], F16, tag="yt")
                        nc.vector.tensor_copy(yt[:, :], psum_t[:, :])
                        psum_o = ppool.tile([128, 128], F32, space="PSUM",
                                            tag="pO")
                        nc.tensor.matmul(out=psum_o[:, :], lhsT=yt[:, :],
                                         rhs=w_sb[:, :], start=True, stop=True)
                        o = smpool.tile([128, 128], F16, tag="o")
                        nc.vector.tensor_copy(o[:, :], psum_o[:, :])
                        nc.sync.dma_start(out[t * 128:(t + 1) * 128, :], o[:, :])

    nc.compile()
    return nc


# ---------------------------------------------------------------------------
# Entry point
# ---------------------------------------------------------------------------

_CACHE = {}


def _get_program_and_arrays(X, weights, row_pointers, column_index, degrees,
                            repeats=1):
    n_nodes, d = X.shape
    cores, rows_per_core, n_total_rows = shard_edges(
        row_pointers, column_index, degrees, NCORES)
    meta, arrays = prep_all(cores, rows_per_core, n_nodes, gt=GT, nsub=NSUB)
    nc = build_gcn(meta, n_nodes, d, num_devices=NCORES, repeats=repeats)
    return nc, meta, arrays, rows_per_core, n_total_rows


def kernel(X, weights, row_pointers, column_index, degrees):
    X = np.asarray(X)
    weights = np.asarray(weights)
    row_pointers = np.asarray(row_pointers)
    column_index = np.asarray(column_index)
    degrees = np.asarray(degrees)
    n_nodes, d = X.shape
    assert d == 128 and weights.shape == (128, 128)

    nc, meta, arrays, rows_per_core, n_total_rows = _get_program_and_arrays(
        X, weights, row_pointers, column_index, degrees)

    Xf = np.ascontiguousarray(X.astype(np.float16))
    Wf = np.ascontiguousarray(weights.astype(np.float16))
    in_maps = [{"x": Xf, "w": Wf, **arrays[k]} for k in range(NCORES)]
    res = run_bass_kernel_spmd(nc, in_maps, list(range(NCORES)), trace=False)

    pieces = []
    for k in range(NCORES):
        r_lo = min(k * rows_per_core, n_total_rows)
        r_hi = min(r_lo + rows_per_core, n_total_rows)
        pieces.append(res.results[k]["out"][:r_hi - r_lo])
    return np.concatenate(pieces).astype(np.float16)
